# revision 1
# baseline (speedup 1.0000x reference)
"""JSD-of-KDE kernel for Trainium2 (8 NeuronCores, data-parallel).

Math: s[j] = sum_i exp(-||grid_j - x_i||^2 / (2 h^2)),  h = 0.05
      p = s / sum(s);  out = JSD(p, q)   (q = target_distribution as given)

Device mapping per core (data_points sharded 8 ways, grid replicated):
  - d^2 computed on TensorE as a K=4 matmul:
      lhsT rows [gx, gy, |g|^2, 1]  (stationary, 128 grid pts / block)
      rhs  rows [-2x, -2y, 1, |x|^2] (moving, 512 data cols / matmul)
  - exp(-200 * d^2) fused with the data-axis reduction on ScalarE:
      activation(Exp, scale=-200, accum_out=[128,1])
  - partial [G] accumulator AllReduce'd across the 8 cores, then the
    normalization + JSD tail runs (redundantly) on every core.
"""

import sys

sys.path.insert(0, "/opt/trn_rl_repo")
import numpy as np

N_CORES = 8
N, G = 100000, 20000
N_PER = N // N_CORES            # 12500
N_PAD = 12800                   # = 6*2048 + 512, multiple of 512
G_PAD = 20096                   # = 157 * 128
GB = G_PAD // 128               # 157 grid blocks
CHUNK = 2048                    # psum tile free size (4 banks)
CHUNKS = [(i * CHUNK, CHUNK) for i in range(6)] + [(6 * CHUNK, 512)]
NCH = len(CHUNKS)               # 7
SCALE = -200.0                  # -1 / (2 * 0.05^2)
TINY = 1e-30
DATA_PAD_VAL = 100.0            # sentinel for padded data points
GRID_PAD_VAL = -100.0           # sentinel for padded grid points

_CACHE = {}
LAST_RESULT = None              # BassKernelResults of the most recent run


def _build():
    from concourse import bacc, tile, mybir

    dt = mybir.dt.float32
    AF = mybir.ActivationFunctionType
    nc = bacc.Bacc(
        "TRN2", target_bir_lowering=False, debug=False, num_devices=N_CORES
    )
    data_ap = nc.dram_tensor("data_aug", [4, N_PAD], dt, kind="ExternalInput").ap()
    grid_ap = nc.dram_tensor("grid_aug", [4, G_PAD], dt, kind="ExternalInput").ap()
    q_ap = nc.dram_tensor("q2d", [128, GB], dt, kind="ExternalInput").ap()
    out_ap = nc.dram_tensor("out", [1, 1], dt, kind="ExternalOutput").ap()

    with tile.TileContext(nc) as tc:
        with (
            tc.tile_pool(name="const", bufs=1) as cpool,
            tc.tile_pool(name="psum", bufs=2, space="PSUM") as ppool,
            tc.tile_pool(name="dram", bufs=1, space="DRAM") as dpool,
            tc.tile_pool(name="work", bufs=1) as wpool,
        ):
            data_sb = cpool.tile([4, N_PAD], dt, tag="data")
            grid_sb = cpool.tile([4, G_PAD], dt, tag="grid")
            q_sb = cpool.tile([128, GB], dt, tag="q")
            part = cpool.tile([128, GB * NCH], dt, tag="part")
            nc.sync.dma_start(data_sb[:], data_ap[:])
            nc.sync.dma_start(grid_sb[:], grid_ap[:])
            nc.sync.dma_start(q_sb[:], q_ap[:])

            for g in range(GB):
                lhsT = grid_sb[:, g * 128 : (g + 1) * 128]
                for ci, (off, sz) in enumerate(CHUNKS):
                    pt = ppool.tile([128, CHUNK], dt, tag="big")
                    for j in range(sz // 512):
                        nc.tensor.matmul(
                            pt[:, j * 512 : (j + 1) * 512],
                            lhsT=lhsT,
                            rhs=data_sb[:, off + j * 512 : off + (j + 1) * 512],
                            start=True,
                            stop=True,
                        )
                    k = g * NCH + ci
                    nc.scalar.activation(
                        pt[:, :sz],
                        pt[:, :sz],
                        AF.Exp,
                        scale=SCALE,
                        accum_out=part[:, k : k + 1],
                    )

            # local reduce over the NCH chunk partials -> s_loc [128, GB]
            s_loc = wpool.tile([128, GB], dt, tag="sloc")
            nc.vector.reduce_sum(
                s_loc[:],
                part[:].rearrange("p (g c) -> p g c", c=NCH),
                axis=mybir.AxisListType.X,
            )

            # all-reduce the [G] accumulator across the 8 cores
            cc_in = dpool.tile([128, GB], dt, tag="ccin")
            cc_out = dpool.tile([128, GB], dt, tag="ccout")
            nc.sync.dma_start(cc_in[:], s_loc[:])
            nc.gpsimd.collective_compute(
                "AllReduce",
                mybir.AluOpType.add,
                replica_groups=[list(range(N_CORES))],
                ins=[cc_in.opt()],
                outs=[cc_out.opt()],
            )
            s_tot = wpool.tile([128, GB], dt, tag="stot")
            nc.sync.dma_start(s_tot[:], cc_out[:])

            # ---- tail: p = s/S, JSD(p, q) ----
            ones_col = cpool.tile([128, 1], dt, tag="onec")
            ones_row = cpool.tile([1, 128], dt, tag="oner")
            nc.vector.memset(ones_col[:], 1.0)
            nc.vector.memset(ones_row[:], 1.0)

            colsum = ppool.tile([1, GB], dt, tag="big")
            nc.tensor.matmul(
                colsum[:], lhsT=ones_col[:], rhs=s_tot[:], start=True, stop=True
            )
            S_sb = wpool.tile([1, 1], dt, tag="Ssb")
            nc.vector.reduce_sum(S_sb[:], colsum[:], axis=mybir.AxisListType.X)
            Sinv = wpool.tile([1, 1], dt, tag="Sinv")
            nc.vector.reciprocal(Sinv[:], S_sb[:])
            binv = ppool.tile([128, 1], dt, tag="big")
            nc.tensor.matmul(
                binv[:], lhsT=ones_row[:], rhs=Sinv[:], start=True, stop=True
            )
            sinv_b = wpool.tile([128, 1], dt, tag="sinvb")
            nc.vector.tensor_copy(sinv_b[:], binv[:])

            p_sb = wpool.tile([128, GB], dt, tag="p")
            nc.vector.tensor_scalar_mul(p_sb[:], s_tot[:], sinv_b[:])
            m_sb = wpool.tile([128, GB], dt, tag="m")
            nc.vector.tensor_add(m_sb[:], p_sb[:], q_sb[:])
            nc.vector.tensor_scalar_mul(m_sb[:], m_sb[:], 0.5)

            pm = wpool.tile([128, GB], dt, tag="pm")
            qm = wpool.tile([128, GB], dt, tag="qm")
            mm = wpool.tile([128, GB], dt, tag="mm")
            nc.vector.tensor_scalar_max(pm[:], p_sb[:], TINY)
            nc.vector.tensor_scalar_max(qm[:], q_sb[:], TINY)
            nc.vector.tensor_scalar_max(mm[:], m_sb[:], TINY)
            lp = wpool.tile([128, GB], dt, tag="lp")
            lq = wpool.tile([128, GB], dt, tag="lq")
            lm = wpool.tile([128, GB], dt, tag="lm")
            nc.scalar.activation(lp[:], pm[:], AF.Ln)
            nc.scalar.activation(lq[:], qm[:], AF.Ln)
            nc.scalar.activation(lm[:], mm[:], AF.Ln)
            # terms = p*(ln p - ln m) + q*(ln q - ln m); exact 0 where p/q == 0
            nc.vector.tensor_sub(lp[:], lp[:], lm[:])
            nc.vector.tensor_sub(lq[:], lq[:], lm[:])
            nc.vector.tensor_mul(lp[:], lp[:], p_sb[:])
            nc.vector.tensor_mul(lq[:], lq[:], q_sb[:])
            nc.vector.tensor_add(lp[:], lp[:], lq[:])

            row_tot = wpool.tile([128, 1], dt, tag="rowt")
            nc.vector.reduce_sum(row_tot[:], lp[:], axis=mybir.AxisListType.X)
            fin = ppool.tile([1, 1], dt, tag="big")
            nc.tensor.matmul(
                fin[:], lhsT=ones_col[:], rhs=row_tot[:], start=True, stop=True
            )
            out_sb = wpool.tile([1, 1], dt, tag="outsb")
            nc.scalar.mul(out_sb[:], fin[:], 0.5)
            nc.sync.dma_start(out_ap[:], out_sb[:])

    nc.compile()
    return nc


def _prep_inputs(data_points, grid, target_distribution):
    data = np.ascontiguousarray(data_points, dtype=np.float32)
    grd = np.ascontiguousarray(grid, dtype=np.float32)
    q = np.ascontiguousarray(target_distribution, dtype=np.float32)

    # grid_aug [4, G_PAD]: rows [gx, gy, |g|^2, 1]; pad with far sentinel
    grid_aug = np.empty((4, G_PAD), np.float32)
    grid_aug[0, :G] = grd[:, 0]
    grid_aug[1, :G] = grd[:, 1]
    grid_aug[2, :G] = grd[:, 0] ** 2 + grd[:, 1] ** 2
    grid_aug[3, :] = 1.0
    grid_aug[0, G:] = GRID_PAD_VAL
    grid_aug[1, G:] = GRID_PAD_VAL
    grid_aug[2, G:] = 2.0 * GRID_PAD_VAL**2

    # q2d [128, GB]: q2d[p, g] = q_padded[g*128 + p]
    q_pad = np.zeros(G_PAD, np.float32)
    q_pad[:G] = q
    q2d = np.ascontiguousarray(q_pad.reshape(GB, 128).T)

    in_maps = []
    for c in range(N_CORES):
        sl = data[c * N_PER : (c + 1) * N_PER]
        x = np.full(N_PAD, DATA_PAD_VAL, np.float32)
        y = np.full(N_PAD, DATA_PAD_VAL, np.float32)
        x[:N_PER] = sl[:, 0]
        y[:N_PER] = sl[:, 1]
        da = np.empty((4, N_PAD), np.float32)
        da[0] = -2.0 * x
        da[1] = -2.0 * y
        da[2] = 1.0
        da[3] = x**2 + y**2
        in_maps.append({"data_aug": da, "grid_aug": grid_aug, "q2d": q2d})
    return in_maps


def kernel(data_points, grid, target_distribution):
    global LAST_RESULT
    from concourse.bass_utils import run_bass_kernel_spmd

    if "nc" not in _CACHE:
        _CACHE["nc"] = _build()
    nc = _CACHE["nc"]
    in_maps = _prep_inputs(data_points, grid, target_distribution)
    res = run_bass_kernel_spmd(nc, in_maps, list(range(N_CORES)))
    LAST_RESULT = res
    out = np.float32(res.results[0]["out"][0, 0])
    return np.asarray(out, dtype=np.float32).reshape(())


# revision 5
# speedup vs baseline: 2.9509x; 2.9509x over previous
"""JSD-of-KDE kernel for Trainium2 (8 NeuronCores, data-parallel).

Math: s[j] = sum_i exp(-||grid_j - x_i||^2 / (2 h^2)),  h = 0.05
      p = s / sum(s);  out = JSD(p, q)   (q = target_distribution as given)

Device mapping per core (data_points sharded 8 ways, grid replicated):
  - d^2 - |g|^2 = -2 g.x + |x|^2 computed on TensorE as a K=16 bf16 matmul.
    fp32 matmul runs LOW+HIGH dual-pass at ~2x the bf16 cost, so instead
    each fp32 coordinate is split 3-way into bf16 limbs (h+m+l); the 12
    cross products >= 2^-26 plus a 3-way-split |x|^2 row give max 5.4e-5
    relative error on the kernel sums (validated offline vs fp64).
    Extra K-rows are free: PE stream time depends on N (moving cols), not K.
  - |g|^2 rides the ScalarE activation bias (exact fp32, per partition):
      exp(scale*in + bias), scale=-200, bias=-200*|g|^2
    fused with the data-axis reduction via accum_out.
  - partial [G] accumulator AllReduce'd across the 8 cores, then the
    normalization + JSD tail runs (redundantly) on every core.
"""

import sys

sys.path.insert(0, "/opt/trn_rl_repo")
import numpy as np

N_CORES = 8
N, G = 100000, 20000
N_PER = N // N_CORES            # 12500
N_PAD = 12800                   # = 6*2048 + 512, multiple of 512
G_PAD = 20096                   # = 157 * 128
GB = G_PAD // 128               # 157 grid blocks
CHUNK = 2048                    # psum tile free size (4 banks)
CHUNKS = [(i * CHUNK, CHUNK) for i in range(6)] + [(6 * CHUNK, 512)]
NCH = len(CHUNKS)               # 7
SCALE = -200.0                  # -1 / (2 * 0.05^2)
TINY = 1e-30
DATA_PAD_VAL = 100.0            # sentinel for padded data points
GRID_PAD_VAL = -100.0           # sentinel for padded grid points

_CACHE = {}
LAST_RESULT = None              # BassKernelResults of the most recent run


def _build():
    from concourse import bacc, tile, mybir

    dt = mybir.dt.float32
    bt = mybir.dt.bfloat16
    AF = mybir.ActivationFunctionType
    nc = bacc.Bacc(
        "TRN2", target_bir_lowering=False, debug=False, num_devices=N_CORES
    )
    data_ap = nc.dram_tensor("data_aug", [16, N_PAD], bt, kind="ExternalInput").ap()
    grid_ap = nc.dram_tensor("grid_aug", [16, G_PAD], bt, kind="ExternalInput").ap()
    gsq_ap = nc.dram_tensor("gsq2d", [128, GB], dt, kind="ExternalInput").ap()
    q_ap = nc.dram_tensor("q2d", [128, GB], dt, kind="ExternalInput").ap()
    out_ap = nc.dram_tensor("out", [1, 1], dt, kind="ExternalOutput").ap()

    with tile.TileContext(nc) as tc:
        with (
            tc.tile_pool(name="const", bufs=1) as cpool,
            tc.tile_pool(name="psum", bufs=2, space="PSUM") as ppool,
            tc.tile_pool(name="dram", bufs=1, space="DRAM") as dpool,
            tc.tile_pool(name="work", bufs=1) as wpool,
        ):
            data_sb = cpool.tile([16, N_PAD], bt, tag="data")
            grid_sb = cpool.tile([16, G_PAD], bt, tag="grid")
            gsq_sb = cpool.tile([128, GB], dt, tag="gsq")
            q_sb = cpool.tile([128, GB], dt, tag="q")
            part = cpool.tile([128, GB * NCH], dt, tag="part")
            nc.sync.dma_start(data_sb[:], data_ap[:])
            nc.sync.dma_start(grid_sb[:], grid_ap[:])
            nc.sync.dma_start(gsq_sb[:], gsq_ap[:])
            nc.sync.dma_start(q_sb[:], q_ap[:])

            for g in range(GB):
                lhsT = grid_sb[:, g * 128 : (g + 1) * 128]
                for ci, (off, sz) in enumerate(CHUNKS):
                    pt = ppool.tile([128, CHUNK], dt, tag="big")
                    for j in range(sz // 512):
                        nc.tensor.matmul(
                            pt[:, j * 512 : (j + 1) * 512],
                            lhsT=lhsT,
                            rhs=data_sb[:, off + j * 512 : off + (j + 1) * 512],
                            start=True,
                            stop=True,
                        )
                    k = g * NCH + ci
                    nc.scalar.activation(
                        pt[:, :sz],
                        pt[:, :sz],
                        AF.Exp,
                        scale=SCALE,
                        bias=gsq_sb[:, g : g + 1],
                        accum_out=part[:, k : k + 1],
                    )

            # local reduce over the NCH chunk partials -> s_loc [128, GB]
            s_loc = wpool.tile([128, GB], dt, tag="sloc")
            nc.vector.reduce_sum(
                s_loc[:],
                part[:].rearrange("p (g c) -> p g c", c=NCH),
                axis=mybir.AxisListType.X,
            )

            # all-reduce the [G] accumulator across the 8 cores
            cc_in = dpool.tile([128, GB], dt, tag="ccin")
            cc_out = dpool.tile([128, GB], dt, tag="ccout")
            nc.sync.dma_start(cc_in[:], s_loc[:])
            nc.gpsimd.collective_compute(
                "AllReduce",
                mybir.AluOpType.add,
                replica_groups=[list(range(N_CORES))],
                ins=[cc_in.opt()],
                outs=[cc_out.opt()],
            )
            s_tot = wpool.tile([128, GB], dt, tag="stot")
            nc.sync.dma_start(s_tot[:], cc_out[:])

            # ---- tail: p = s/S, JSD(p, q) ----
            ones_col = cpool.tile([128, 1], dt, tag="onec")
            ones_row = cpool.tile([1, 128], dt, tag="oner")
            nc.vector.memset(ones_col[:], 1.0)
            nc.vector.memset(ones_row[:], 1.0)

            colsum = ppool.tile([1, GB], dt, tag="big")
            nc.tensor.matmul(
                colsum[:], lhsT=ones_col[:], rhs=s_tot[:], start=True, stop=True
            )
            S_sb = wpool.tile([1, 1], dt, tag="Ssb")
            nc.vector.reduce_sum(S_sb[:], colsum[:], axis=mybir.AxisListType.X)
            Sinv = wpool.tile([1, 1], dt, tag="Sinv")
            nc.vector.reciprocal(Sinv[:], S_sb[:])
            binv = ppool.tile([128, 1], dt, tag="big")
            nc.tensor.matmul(
                binv[:], lhsT=ones_row[:], rhs=Sinv[:], start=True, stop=True
            )
            sinv_b = wpool.tile([128, 1], dt, tag="sinvb")
            nc.vector.tensor_copy(sinv_b[:], binv[:])

            p_sb = wpool.tile([128, GB], dt, tag="p")
            nc.vector.tensor_scalar_mul(p_sb[:], s_tot[:], sinv_b[:])
            m_sb = wpool.tile([128, GB], dt, tag="m")
            nc.vector.tensor_add(m_sb[:], p_sb[:], q_sb[:])
            nc.vector.tensor_scalar_mul(m_sb[:], m_sb[:], 0.5)

            pm = wpool.tile([128, GB], dt, tag="pm")
            qm = wpool.tile([128, GB], dt, tag="qm")
            mm = wpool.tile([128, GB], dt, tag="mm")
            nc.vector.tensor_scalar_max(pm[:], p_sb[:], TINY)
            nc.vector.tensor_scalar_max(qm[:], q_sb[:], TINY)
            nc.vector.tensor_scalar_max(mm[:], m_sb[:], TINY)
            lp = wpool.tile([128, GB], dt, tag="lp")
            lq = wpool.tile([128, GB], dt, tag="lq")
            lm = wpool.tile([128, GB], dt, tag="lm")
            nc.scalar.activation(lp[:], pm[:], AF.Ln)
            nc.scalar.activation(lq[:], qm[:], AF.Ln)
            nc.scalar.activation(lm[:], mm[:], AF.Ln)
            # terms = p*(ln p - ln m) + q*(ln q - ln m); exact 0 where p/q == 0
            nc.vector.tensor_sub(lp[:], lp[:], lm[:])
            nc.vector.tensor_sub(lq[:], lq[:], lm[:])
            nc.vector.tensor_mul(lp[:], lp[:], p_sb[:])
            nc.vector.tensor_mul(lq[:], lq[:], q_sb[:])
            nc.vector.tensor_add(lp[:], lp[:], lq[:])

            row_tot = wpool.tile([128, 1], dt, tag="rowt")
            nc.vector.reduce_sum(row_tot[:], lp[:], axis=mybir.AxisListType.X)
            fin = ppool.tile([1, 1], dt, tag="big")
            nc.tensor.matmul(
                fin[:], lhsT=ones_col[:], rhs=row_tot[:], start=True, stop=True
            )
            out_sb = wpool.tile([1, 1], dt, tag="outsb")
            nc.scalar.mul(out_sb[:], fin[:], 0.5)
            nc.sync.dma_start(out_ap[:], out_sb[:])

    nc.compile()
    return nc


def _split3(v):
    """3-way bf16 limb split: v ~= h + m + l with each limb bf16."""
    import ml_dtypes

    bf = ml_dtypes.bfloat16
    h = v.astype(bf).astype(np.float32)
    r = v - h
    m = r.astype(bf).astype(np.float32)
    l = (r - m).astype(bf).astype(np.float32)
    return h, m, l


def _prep_inputs(data_points, grid, target_distribution):
    import ml_dtypes

    bf = ml_dtypes.bfloat16
    data = np.ascontiguousarray(data_points, dtype=np.float32)
    grd = np.ascontiguousarray(grid, dtype=np.float32)
    q = np.ascontiguousarray(target_distribution, dtype=np.float32)

    # grid coordinates, padded with far sentinel, split into bf16 limbs
    gx = np.full(G_PAD, GRID_PAD_VAL, np.float32)
    gy = np.full(G_PAD, GRID_PAD_VAL, np.float32)
    gx[:G] = grd[:, 0]
    gy[:G] = grd[:, 1]
    ghx, gmx, glx = _split3(gx)
    ghy, gmy, gly = _split3(gy)
    # 16 stationary rows pairing with the moving rows below (row15 zero)
    grid_aug = np.zeros((16, G_PAD), np.float32)
    grid_aug[0], grid_aug[1] = ghx, ghy          # . -2xh
    grid_aug[2], grid_aug[3] = ghx, ghy          # . -2xm
    grid_aug[4], grid_aug[5] = gmx, gmy          # . -2xh
    grid_aug[6], grid_aug[7] = ghx, ghy          # . -2xl
    grid_aug[8], grid_aug[9] = glx, gly          # . -2xh
    grid_aug[10], grid_aug[11] = gmx, gmy        # . -2xm
    grid_aug[12:15] = 1.0                        # . x2 h/m/l
    grid_aug = grid_aug.astype(bf)

    # gsq2d [128, GB]: activation bias = -200 * |g|^2 (exact fp32)
    gsq = (-200.0 * (gx * gx + gy * gy)).astype(np.float32)
    gsq2d = np.ascontiguousarray(gsq.reshape(GB, 128).T)

    # q2d [128, GB]: q2d[p, g] = q_padded[g*128 + p]
    q_pad = np.zeros(G_PAD, np.float32)
    q_pad[:G] = q
    q2d = np.ascontiguousarray(q_pad.reshape(GB, 128).T)

    in_maps = []
    for c in range(N_CORES):
        sl = data[c * N_PER : (c + 1) * N_PER]
        x = np.full(N_PAD, DATA_PAD_VAL, np.float32)
        y = np.full(N_PAD, DATA_PAD_VAL, np.float32)
        x[:N_PER] = sl[:, 0]
        y[:N_PER] = sl[:, 1]
        xh, xm, xl = _split3(x)
        yh, ym, yl = _split3(y)
        x2h, x2m, x2l = _split3(x * x + y * y)
        da = np.zeros((16, N_PAD), np.float32)
        da[0], da[1] = -2 * xh, -2 * yh
        da[2], da[3] = -2 * xm, -2 * ym
        da[4], da[5] = -2 * xh, -2 * yh
        da[6], da[7] = -2 * xl, -2 * yl
        da[8], da[9] = -2 * xh, -2 * yh
        da[10], da[11] = -2 * xm, -2 * ym
        da[12], da[13], da[14] = x2h, x2m, x2l
        da = da.astype(bf)
        in_maps.append(
            {"data_aug": da, "grid_aug": grid_aug, "gsq2d": gsq2d, "q2d": q2d}
        )
    return in_maps


def kernel(data_points, grid, target_distribution):
    global LAST_RESULT
    from concourse.bass_utils import run_bass_kernel_spmd

    if "nc" not in _CACHE:
        _CACHE["nc"] = _build()
    nc = _CACHE["nc"]
    in_maps = _prep_inputs(data_points, grid, target_distribution)
    res = run_bass_kernel_spmd(nc, in_maps, list(range(N_CORES)))
    LAST_RESULT = res
    out = np.float32(res.results[0]["out"][0, 0])
    return np.asarray(out, dtype=np.float32).reshape(())


# revision 6
# speedup vs baseline: 3.0952x; 1.0489x over previous
"""JSD-of-KDE kernel for Trainium2 (8 NeuronCores, data-parallel).

Math: s[j] = sum_i exp(-||grid_j - x_i||^2 / (2 h^2)),  h = 0.05
      p = s / sum(s);  out = JSD(p, q)   (q = target_distribution as given)

Device mapping per core (data_points sharded 8 ways, grid replicated):
  - d^2 - |g|^2 = -2 g.x + |x|^2 computed on TensorE as a K=16 bf16 matmul.
    fp32 matmul runs LOW+HIGH dual-pass at ~2x the bf16 cost, so instead
    each fp32 coordinate is split 3-way into bf16 limbs (h+m+l); the 12
    cross products >= 2^-26 plus a 3-way-split |x|^2 row give max 5.4e-5
    relative error on the kernel sums (validated offline vs fp64).
    Extra K-rows are free: PE stream time depends on N (moving cols), not K.
  - |g|^2 rides the ScalarE activation bias (exact fp32, per partition):
      exp(scale*in + bias), scale=-200, bias=-200*|g|^2
    fused with the data-axis reduction via accum_out.
  - partial [G] accumulator AllReduce'd across the 8 cores, then the
    normalization + JSD tail runs (redundantly) on every core.
"""

import sys

sys.path.insert(0, "/opt/trn_rl_repo")
import numpy as np

N_CORES = 8
N, G = 100000, 20000
N_PER = N // N_CORES            # 12500
N_PAD = 12800                   # = 6*2048 + 512, multiple of 512
G_PAD = 20096                   # = 157 * 128
GB = G_PAD // 128               # 157 grid blocks
CHUNK = 2048                    # psum tile free size (4 banks)
CHUNKS = [(i * CHUNK, CHUNK) for i in range(6)] + [(6 * CHUNK, 512)]
NCH = len(CHUNKS)               # 7
SCALE = -200.0                  # -1 / (2 * 0.05^2)
TINY = 1e-30
DATA_PAD_VAL = 100.0            # sentinel for padded data points
GRID_PAD_VAL = -100.0           # sentinel for padded grid points

_CACHE = {}
LAST_RESULT = None              # BassKernelResults of the most recent run


def _build():
    from concourse import bacc, tile, mybir

    dt = mybir.dt.float32
    bt = mybir.dt.bfloat16
    AF = mybir.ActivationFunctionType
    nc = bacc.Bacc(
        "TRN2", target_bir_lowering=False, debug=False, num_devices=N_CORES
    )
    data_ap = nc.dram_tensor("data_aug", [16, N_PAD], bt, kind="ExternalInput").ap()
    grid_ap = nc.dram_tensor("grid_aug", [16, G_PAD], bt, kind="ExternalInput").ap()
    gsq_ap = nc.dram_tensor("gsq2d", [128, GB], dt, kind="ExternalInput").ap()
    q_ap = nc.dram_tensor("q2d", [128, GB], dt, kind="ExternalInput").ap()
    out_ap = nc.dram_tensor("out", [1, 1], dt, kind="ExternalOutput").ap()

    with tile.TileContext(nc) as tc:
        with (
            tc.tile_pool(name="const", bufs=1) as cpool,
            tc.tile_pool(name="psum", bufs=2, space="PSUM") as ppool,
            tc.tile_pool(name="dram", bufs=1, space="DRAM") as dpool,
            tc.tile_pool(name="work", bufs=1) as wpool,
        ):
            data_sb = cpool.tile([16, N_PAD], bt, tag="data")
            grid_sb = cpool.tile([16, G_PAD], bt, tag="grid")
            gsq_sb = cpool.tile([128, GB], dt, tag="gsq")
            q_sb = cpool.tile([128, GB], dt, tag="q")
            part = cpool.tile([128, GB * NCH], dt, tag="part")
            nc.sync.dma_start(data_sb[:], data_ap[:])
            nc.sync.dma_start(grid_sb[:], grid_ap[:])
            nc.sync.dma_start(gsq_sb[:], gsq_ap[:])
            nc.sync.dma_start(q_sb[:], q_ap[:])

            with tc.tile_pool(name="scratch", bufs=3) as spool:
                for g in range(GB):
                    lhsT = grid_sb[:, g * 128 : (g + 1) * 128]
                    for ci, (off, sz) in enumerate(CHUNKS):
                        pt = ppool.tile([128, CHUNK], dt, tag="big")
                        for j in range(sz // 512):
                            nc.tensor.matmul(
                                pt[:, j * 512 : (j + 1) * 512],
                                lhsT=lhsT,
                                rhs=data_sb[:, off + j * 512 : off + (j + 1) * 512],
                                start=True,
                                stop=True,
                            )
                        k = g * NCH + ci
                        ex = spool.tile([128, CHUNK], dt, tag="ex")
                        nc.scalar.activation(
                            ex[:, :sz],
                            pt[:, :sz],
                            AF.Exp,
                            scale=SCALE,
                            bias=gsq_sb[:, g : g + 1],
                        )
                        nc.vector.reduce_sum(
                            part[:, k : k + 1],
                            ex[:, :sz],
                            axis=mybir.AxisListType.X,
                        )

            # local reduce over the NCH chunk partials -> s_loc [128, GB]
            s_loc = wpool.tile([128, GB], dt, tag="sloc")
            nc.vector.reduce_sum(
                s_loc[:],
                part[:].rearrange("p (g c) -> p g c", c=NCH),
                axis=mybir.AxisListType.X,
            )

            # all-reduce the [G] accumulator across the 8 cores
            cc_in = dpool.tile([128, GB], dt, tag="ccin")
            cc_out = dpool.tile([128, GB], dt, tag="ccout")
            nc.sync.dma_start(cc_in[:], s_loc[:])
            nc.gpsimd.collective_compute(
                "AllReduce",
                mybir.AluOpType.add,
                replica_groups=[list(range(N_CORES))],
                ins=[cc_in.opt()],
                outs=[cc_out.opt()],
            )
            s_tot = wpool.tile([128, GB], dt, tag="stot")
            nc.sync.dma_start(s_tot[:], cc_out[:])

            # ---- tail: p = s/S, JSD(p, q) ----
            ones_col = cpool.tile([128, 1], dt, tag="onec")
            ones_row = cpool.tile([1, 128], dt, tag="oner")
            nc.vector.memset(ones_col[:], 1.0)
            nc.vector.memset(ones_row[:], 1.0)

            colsum = ppool.tile([1, GB], dt, tag="big")
            nc.tensor.matmul(
                colsum[:], lhsT=ones_col[:], rhs=s_tot[:], start=True, stop=True
            )
            S_sb = wpool.tile([1, 1], dt, tag="Ssb")
            nc.vector.reduce_sum(S_sb[:], colsum[:], axis=mybir.AxisListType.X)
            Sinv = wpool.tile([1, 1], dt, tag="Sinv")
            nc.vector.reciprocal(Sinv[:], S_sb[:])
            binv = ppool.tile([128, 1], dt, tag="big")
            nc.tensor.matmul(
                binv[:], lhsT=ones_row[:], rhs=Sinv[:], start=True, stop=True
            )
            sinv_b = wpool.tile([128, 1], dt, tag="sinvb")
            nc.vector.tensor_copy(sinv_b[:], binv[:])

            p_sb = wpool.tile([128, GB], dt, tag="p")
            nc.vector.tensor_scalar_mul(p_sb[:], s_tot[:], sinv_b[:])
            m_sb = wpool.tile([128, GB], dt, tag="m")
            nc.vector.tensor_add(m_sb[:], p_sb[:], q_sb[:])
            nc.vector.tensor_scalar_mul(m_sb[:], m_sb[:], 0.5)

            pm = wpool.tile([128, GB], dt, tag="pm")
            qm = wpool.tile([128, GB], dt, tag="qm")
            mm = wpool.tile([128, GB], dt, tag="mm")
            nc.vector.tensor_scalar_max(pm[:], p_sb[:], TINY)
            nc.vector.tensor_scalar_max(qm[:], q_sb[:], TINY)
            nc.vector.tensor_scalar_max(mm[:], m_sb[:], TINY)
            lp = wpool.tile([128, GB], dt, tag="lp")
            lq = wpool.tile([128, GB], dt, tag="lq")
            lm = wpool.tile([128, GB], dt, tag="lm")
            nc.scalar.activation(lp[:], pm[:], AF.Ln)
            nc.scalar.activation(lq[:], qm[:], AF.Ln)
            nc.scalar.activation(lm[:], mm[:], AF.Ln)
            # terms = p*(ln p - ln m) + q*(ln q - ln m); exact 0 where p/q == 0
            nc.vector.tensor_sub(lp[:], lp[:], lm[:])
            nc.vector.tensor_sub(lq[:], lq[:], lm[:])
            nc.vector.tensor_mul(lp[:], lp[:], p_sb[:])
            nc.vector.tensor_mul(lq[:], lq[:], q_sb[:])
            nc.vector.tensor_add(lp[:], lp[:], lq[:])

            row_tot = wpool.tile([128, 1], dt, tag="rowt")
            nc.vector.reduce_sum(row_tot[:], lp[:], axis=mybir.AxisListType.X)
            fin = ppool.tile([1, 1], dt, tag="big")
            nc.tensor.matmul(
                fin[:], lhsT=ones_col[:], rhs=row_tot[:], start=True, stop=True
            )
            out_sb = wpool.tile([1, 1], dt, tag="outsb")
            nc.scalar.mul(out_sb[:], fin[:], 0.5)
            nc.sync.dma_start(out_ap[:], out_sb[:])

    nc.compile()
    return nc


def _split3(v):
    """3-way bf16 limb split: v ~= h + m + l with each limb bf16."""
    import ml_dtypes

    bf = ml_dtypes.bfloat16
    h = v.astype(bf).astype(np.float32)
    r = v - h
    m = r.astype(bf).astype(np.float32)
    l = (r - m).astype(bf).astype(np.float32)
    return h, m, l


def _prep_inputs(data_points, grid, target_distribution):
    import ml_dtypes

    bf = ml_dtypes.bfloat16
    data = np.ascontiguousarray(data_points, dtype=np.float32)
    grd = np.ascontiguousarray(grid, dtype=np.float32)
    q = np.ascontiguousarray(target_distribution, dtype=np.float32)

    # grid coordinates, padded with far sentinel, split into bf16 limbs
    gx = np.full(G_PAD, GRID_PAD_VAL, np.float32)
    gy = np.full(G_PAD, GRID_PAD_VAL, np.float32)
    gx[:G] = grd[:, 0]
    gy[:G] = grd[:, 1]
    ghx, gmx, glx = _split3(gx)
    ghy, gmy, gly = _split3(gy)
    # 16 stationary rows pairing with the moving rows below (row15 zero)
    grid_aug = np.zeros((16, G_PAD), np.float32)
    grid_aug[0], grid_aug[1] = ghx, ghy          # . -2xh
    grid_aug[2], grid_aug[3] = ghx, ghy          # . -2xm
    grid_aug[4], grid_aug[5] = gmx, gmy          # . -2xh
    grid_aug[6], grid_aug[7] = ghx, ghy          # . -2xl
    grid_aug[8], grid_aug[9] = glx, gly          # . -2xh
    grid_aug[10], grid_aug[11] = gmx, gmy        # . -2xm
    grid_aug[12:15] = 1.0                        # . x2 h/m/l
    grid_aug = grid_aug.astype(bf)

    # gsq2d [128, GB]: activation bias = -200 * |g|^2 (exact fp32)
    gsq = (-200.0 * (gx * gx + gy * gy)).astype(np.float32)
    gsq2d = np.ascontiguousarray(gsq.reshape(GB, 128).T)

    # q2d [128, GB]: q2d[p, g] = q_padded[g*128 + p]
    q_pad = np.zeros(G_PAD, np.float32)
    q_pad[:G] = q
    q2d = np.ascontiguousarray(q_pad.reshape(GB, 128).T)

    in_maps = []
    for c in range(N_CORES):
        sl = data[c * N_PER : (c + 1) * N_PER]
        x = np.full(N_PAD, DATA_PAD_VAL, np.float32)
        y = np.full(N_PAD, DATA_PAD_VAL, np.float32)
        x[:N_PER] = sl[:, 0]
        y[:N_PER] = sl[:, 1]
        xh, xm, xl = _split3(x)
        yh, ym, yl = _split3(y)
        x2h, x2m, x2l = _split3(x * x + y * y)
        da = np.zeros((16, N_PAD), np.float32)
        da[0], da[1] = -2 * xh, -2 * yh
        da[2], da[3] = -2 * xm, -2 * ym
        da[4], da[5] = -2 * xh, -2 * yh
        da[6], da[7] = -2 * xl, -2 * yl
        da[8], da[9] = -2 * xh, -2 * yh
        da[10], da[11] = -2 * xm, -2 * ym
        da[12], da[13], da[14] = x2h, x2m, x2l
        da = da.astype(bf)
        in_maps.append(
            {"data_aug": da, "grid_aug": grid_aug, "gsq2d": gsq2d, "q2d": q2d}
        )
    return in_maps


def kernel(data_points, grid, target_distribution):
    global LAST_RESULT
    from concourse.bass_utils import run_bass_kernel_spmd

    if "nc" not in _CACHE:
        _CACHE["nc"] = _build()
    nc = _CACHE["nc"]
    in_maps = _prep_inputs(data_points, grid, target_distribution)
    res = run_bass_kernel_spmd(nc, in_maps, list(range(N_CORES)))
    LAST_RESULT = res
    out = np.float32(res.results[0]["out"][0, 0])
    return np.asarray(out, dtype=np.float32).reshape(())
